# revision 60
# baseline (speedup 1.0000x reference)
"""Trainium2 Bass kernel for causal multi-head attention (dense transformer).

Problem shapes (hardcoded): x [2,2048,1024], 16 heads x 64 head-dim.
Sharding: data-parallel over batch (2) x tensor-parallel over heads (4/core)
on 8 NeuronCores. Each core computes the partial output (sum over its 4
heads) for one batch element; the host sums the 4 partials per batch and
adds b_O (+ the constant sum_h W_O[h] @ b_V[h] -- b_V shifts every z by a
constant, b_K cancels in softmax, so neither needs device work).

All operands bf16 except the Q/K projections, which run fp8-e4m3 in
DoubleRow perf mode (256-wide contraction per pass, halving projection
matmul time; x scaled by 32 and W_Q/W_K by 1024 into e4m3 range, descaled
in the PSUM evacuation). V / scores / W_O stay bf16 -- fp8 there fails the
accuracy gate. PSUM always accumulates fp32. Other key points:
  - bf16/fp8 host pre-casts halve HBM traffic; no in-flight cast DMAs
  - b_Q is fused into the Q PSUM->SBUF evacuation via ScalarE activation
    bias (per-partition), so no bias matmuls at all
  - QKV projections run chunk-major (contraction-outer) over x^T as it
    streams in; PE warmup matmuls run on a memset tile with no DMA deps
  - scores are S^T[k,q] strips (k on partitions) with the contraction
    zero-padded 64->128 (keeps the PE HAM clock gate warm / 2.4GHz);
    exp fused with PSUM evacuation on ScalarE (bf16 out); causal mask is
    a 0/1 bf16 multiply on the diagonal block (DVE)
  - AV uses V augmented with a ones column so the softmax denominator
    falls out of the same matmul; strips software-pipelined depth 4
  - the output projection is interleaved into the attention strip stream
    (q-chunks 0/1 during the hf=1 strips, 2/3 right after) so the PE
    never idles long enough for the HAM clock gate to throttle and the
    output DMA overlaps compute
"""

import sys

if "/opt/trn_rl_repo" not in sys.path:
    sys.path.insert(0, "/opt/trn_rl_repo")

import numpy as np
import ml_dtypes

B, S, D = 2, 2048, 1024
H, DH = 16, 64
NCORES = 8
NH = 4            # heads per core
KCH = D // 128    # contraction chunks over model dim
CP = D // 256     # fp8 DoubleRow chunk pairs
NT = S // 128     # 128-row tiles over sequence
P = 128

# fp8-e4m3 DoubleRow Q/K projections (scores only; V/W_O stay bf16).
# Host-simulated end-to-end rel err 1.25e-2 vs the 2e-2 gate.
FP8QK = True
SX = 32.0         # x pre-scale into e4m3 range
SW = 1024.0       # W_Q/W_K pre-scale

_CACHE = {}


def _build_nc():
    import concourse.tile as tile
    from concourse import bacc, mybir

    f32 = mybir.dt.float32
    bf16 = mybir.dt.bfloat16
    fp8 = mybir.dt.float8e4
    Exp = mybir.ActivationFunctionType.Exp
    Ident = mybir.ActivationFunctionType.Identity
    mult = mybir.AluOpType.mult
    DR = mybir.MatmulPerfMode.DoubleRow

    nc = bacc.Bacc("TRN2", target_bir_lowering=False, debug=False,
                   num_devices=NCORES)

    xt_d = nc.dram_tensor("xt", [D, S], bf16, kind="ExternalInput").ap()
    if FP8QK:
        xq_d = nc.dram_tensor("xq", [P, CP * S * 2], fp8, kind="ExternalInput").ap()
        wq_d = nc.dram_tensor("wq", [P, CP * NH * DH * 2], fp8, kind="ExternalInput").ap()
        wk_d = nc.dram_tensor("wk", [P, CP * NH * DH * 2], fp8, kind="ExternalInput").ap()
    else:
        wq_d = nc.dram_tensor("wq", [P, KCH * NH * DH], bf16, kind="ExternalInput").ap()
        wk_d = nc.dram_tensor("wk", [P, KCH * NH * DH], bf16, kind="ExternalInput").ap()
    wv_d = nc.dram_tensor("wv", [P, KCH * NH * DH], bf16, kind="ExternalInput").ap()
    wo_d = nc.dram_tensor("wo", [P, 2 * D], bf16, kind="ExternalInput").ap()
    bq_d = nc.dram_tensor("bq", [P, 2], f32, kind="ExternalInput").ap()
    tri_d = nc.dram_tensor("tri", [P, P], bf16, kind="ExternalInput").ap()
    out_d = nc.dram_tensor("out", [S, D], f32, kind="ExternalOutput").ap()

    with tile.TileContext(nc) as tc:
        from contextlib import ExitStack

        with ExitStack() as ctx:
            persist = ctx.enter_context(tc.tile_pool(name="persist", bufs=1))

            QT = persist.tile([P, 2, S], bf16)
            KT = persist.tile([P, NH, S], bf16)
            V = persist.tile([P, NT, NH, DH + 1], bf16)
            ZN = persist.tile([P, 2, S], bf16)
            if FP8QK:
                # DoubleRow operands: [K, cp, j, ·] with j the pair dim
                XQ = persist.tile([P, CP, 2, S], fp8)
                WQ = persist.tile([P, CP, 2, NH * DH], fp8)
                WK = persist.tile([P, CP, 2, NH * DH], fp8)
            else:
                WQ = persist.tile([P, KCH, NH * DH], bf16)
                WK = persist.tile([P, KCH, NH * DH], bf16)
            WV = persist.tile([P, KCH, NH * DH], bf16)
            WO = persist.tile([P, 2, D], bf16)
            BQ = persist.tile([P, 2], f32)
            TRI = persist.tile([P, P], bf16)
            WRM = persist.tile([P, P], bf16)

            # ---- t0: memsets (no DMA deps) + input DMA kickoff; Q/K
            # weights lead the two fast HWDGE rings (the gpsimd SWDGE ring
            # is slow to spin up and round-robins across its queued DMAs) ----
            nc.vector.memset(WRM, 0.0)
            nc.vector.memset(KT, 0.0)
            nc.vector.memset(V[:, :, :, DH:DH + 1], 1.0)

            nc.gpsimd.dma_start(BQ, bq_d)
            nc.gpsimd.dma_start(TRI, tri_d)
            nc.gpsimd.dma_start(WV.rearrange("p a b -> p (a b)"), wv_d)
            nc.gpsimd.dma_start(WO.rearrange("p a b -> p (a b)"), wo_d)

            xt_ctx = ctx.enter_context(tc.tile_pool(name="xt", bufs=1))
            XT = [xt_ctx.tile([P, S], bf16, name=f"xt{ch}")
                  for ch in range(KCH)]

            if FP8QK:
                nc.sync.dma_start(WK.rearrange("p a b c -> p (a b c)"), wk_d)
                nc.scalar.dma_start(WQ.rearrange("p a b c -> p (a b c)"), wq_d)
                for cp in range(CP):
                    eng = nc.sync if cp % 2 == 0 else nc.scalar
                    eng.dma_start(
                        XQ[:, cp, :, :],
                        xq_d[:, cp * 2 * S:(cp + 1) * 2 * S])
            else:
                wkr = WK.rearrange("p a b -> p (a b)")
                wqr = WQ.rearrange("p a b -> p (a b)")
                hw = KCH * NH * DH // 2
                nc.sync.dma_start(wkr[:, 0:hw], wk_d[:, 0:hw])
                nc.scalar.dma_start(wqr[:, 0:hw], wq_d[:, 0:hw])
                nc.sync.dma_start(wkr[:, hw:], wk_d[:, hw:])
                nc.scalar.dma_start(wqr[:, hw:], wq_d[:, hw:])
            for ch in range(KCH):
                eng = nc.sync if ch % 2 == 0 else nc.scalar
                eng.dma_start(XT[ch], xt_d[ch * P:(ch + 1) * P, :])

            # ---- PE warmup while input DMAs stream (warms HAM clock) ----
            with tc.tile_pool(name="warm_ps", bufs=1, space="PSUM") as wp:
                wps = wp.tile([P, P], f32)
                for _ in range(24):
                    nc.tensor.matmul(wps, WRM, WRM, start=True, stop=True)

            # ---- phase 1: K projection sweep, then Q sweep (K first so its
            # 16 split-plane evacuations overlap the Q matmuls; Q's 8
            # bias-evacs split across ScalarE/DVE are the only bank-release
            # latency the attention pools then wait on) ----
            add = mybir.AluOpType.add
            dsq = 1.0 / (SX * SW * 8.0) if FP8QK else 1.0   # Q descale (attn
            dsk = 1.0 / (SX * SW) if FP8QK else 1.0         # scale on Q side)
            with tc.tile_pool(name="qk_ps", bufs=8, space="PSUM") as qk_ps:
                for wi, W_ in ((1, WK), (0, WQ)):
                    pst = {}
                    for t in range(2):
                        for qc in range(4):
                            pst[(t, qc)] = qk_ps.tile(
                                [P, 512], f32, tag="qk",
                                name=f"qk{wi}_{t}_{qc}")
                    if FP8QK:
                        for cp in range(CP):
                            for t in range(2):
                                for qc in range(4):
                                    nc.tensor.matmul(
                                        pst[(t, qc)],
                                        W_[:, cp, :, t * P:(t + 1) * P],
                                        XQ[:, cp, :, qc * 512:(qc + 1) * 512],
                                        start=(cp == 0), stop=(cp == CP - 1),
                                        perf_mode=DR)
                    else:
                        for ch in range(KCH):
                            for t in range(2):
                                for qc in range(4):
                                    nc.tensor.matmul(
                                        pst[(t, qc)],
                                        W_[:, ch, t * P:(t + 1) * P],
                                        XT[ch][:, qc * 512:(qc + 1) * 512],
                                        start=(ch == 0), stop=(ch == KCH - 1))
                    for t in range(2):
                        for qc in range(4):
                            sl = slice(qc * 512, (qc + 1) * 512)
                            ps = pst[(t, qc)]
                            if wi == 1:
                                # K: split head halves into zero-padded planes
                                if FP8QK:
                                    nc.scalar.mul(
                                        KT[0:64, 2 * t, sl], ps[0:64, :], dsk)
                                    nc.vector.tensor_scalar_mul(
                                        KT[64:128, 2 * t + 1, sl],
                                        ps[64:128, :], dsk)
                                else:
                                    nc.scalar.copy(
                                        KT[0:64, 2 * t, sl], ps[0:64, :])
                                    nc.vector.tensor_copy(
                                        KT[64:128, 2 * t + 1, sl],
                                        ps[64:128, :])
                            elif t == 0:
                                # Q: evacuate with b_Q fused as bias
                                nc.scalar.activation(
                                    QT[:, t, sl], ps, Ident,
                                    bias=BQ[:, t:t + 1], scale=dsq)
                            elif FP8QK:
                                nc.vector.tensor_scalar(
                                    QT[:, t, sl], ps, dsq, BQ[:, t:t + 1],
                                    mult, add)
                            else:
                                nc.vector.tensor_scalar(
                                    QT[:, t, sl], ps, BQ[:, t:t + 1],
                                    None, add)

            # ---- phase 2: attention strips, V projection folded into the
            # pipeline fill, out-projection interleaved into the stream ----
            with tc.tile_pool(name="esp", bufs=6) as esp, \
                    tc.tile_pool(name="nrm", bufs=4) as nrm, \
                    tc.tile_pool(name="osb", bufs=4) as osb, \
                    tc.tile_pool(name="sc_ps", bufs=2, space="PSUM") as sc_ps, \
                    tc.tile_pool(name="av_ps", bufs=2, space="PSUM") as av_ps:
                avs = {}

                def emit_scores(h, kb, hf):
                    t = h // 2
                    k0 = kb * P
                    hstart = hf * 1024
                    qstart = max(k0, hstart)
                    sps = sc_ps.tile([P, 1024], f32,
                                     name=f"sps_{h}_{kb}_{hf}", tag="sps")
                    ssb = esp.tile([P, 1024], bf16,
                                   name=f"ssb_{h}_{kb}_{hf}", tag="ssb")
                    qpos = qstart
                    while qpos < hstart + 1024:
                        qnext = min(hstart + 1024, (qpos // 512 + 1) * 512)
                        nc.tensor.matmul(
                            sps[:, qpos - hstart:qnext - hstart],
                            KT[:, h, k0:k0 + P],
                            QT[:, t, qpos:qnext],
                            start=True, stop=True)
                        qpos = qnext
                    nc.scalar.activation(
                        ssb[:, qstart - hstart:1024],
                        sps[:, qstart - hstart:1024], Exp)
                    if k0 >= hstart:
                        dsl = slice(k0 - hstart, k0 - hstart + P)
                        nc.vector.tensor_tensor(
                            ssb[:, dsl], ssb[:, dsl], TRI, mult)
                    return ssb

                def emit_norm(h, qc, avq):
                    t, pb = h // 2, (h % 2) * 64
                    rd = nrm.tile([1, 512], f32, tag="rd")
                    nc.vector.tensor_copy(rd, avq[DH:DH + 1, :])
                    rr = nrm.tile([1, 512], f32, tag="rr")
                    nc.vector.reciprocal_approx_fast(out=rr, in_=rd)
                    rdb = nrm.tile([64, 512], f32, tag="rdb")
                    nc.gpsimd.partition_broadcast(rdb, rr)
                    zslc = ZN[pb:pb + 64, t, qc * 512:(qc + 1) * 512]
                    nc.vector.tensor_tensor(zslc, avq[0:DH, :], rdb, mult)

                def emit_av(h, kb, hf, ssb):
                    k0 = kb * P
                    hstart = hf * 1024
                    qstart = max(k0, hstart)
                    if kb == 0:
                        for qc in (2 * hf, 2 * hf + 1):
                            avs[(h, qc)] = av_ps.tile(
                                [DH + 1, 512], f32,
                                tag="av", name=f"av_{h}_{qc}")
                    qpos = qstart
                    while qpos < hstart + 1024:
                        qc = qpos // 512
                        qnext = min(hstart + 1024, (qc + 1) * 512)
                        done = kb == 4 * qc + 3
                        nc.tensor.matmul(
                            avs[(h, qc)][:, qpos - qc * 512:qnext - qc * 512],
                            V[:, kb, h, :],
                            ssb[:, qpos - hstart:qnext - hstart],
                            start=(kb == 0), stop=done)
                        if done:
                            emit_norm(h, qc, avs[(h, qc)])
                        qpos = qnext

                def emit_opunit(pool, qt, dc, evac, dma):
                    ps = pool.tile([P, 512], f32, tag="op",
                                   name=f"op_{qt}_{dc}")
                    for t in range(2):
                        nc.tensor.matmul(
                            ps, ZN[:, t, qt * P:(qt + 1) * P],
                            WO[:, t, dc * 512:(dc + 1) * 512],
                            start=(t == 0), stop=(t == 1))
                    ob = osb.tile([P, 512], f32, tag="osb",
                                  name=f"ob_{qt}_{dc}")
                    if evac == 0:
                        nc.vector.tensor_copy(ob, ps)
                    else:
                        nc.scalar.copy(ob, ps)
                    dma.dma_start(
                        out_d[qt * P:(qt + 1) * P, dc * 512:(dc + 1) * 512],
                        ob)

                strips = [(h, kb, 0) for h in range(NH) for kb in range(8)]
                strips += [(h, kb, 1) for h in range(NH) for kb in range(NT)]

                from collections import deque
                pending = deque()

                def do_strip(sid):
                    ssb = emit_scores(*sid)
                    pending.append((sid, ssb))
                    if len(pending) > 5:
                        psid, pssb = pending.popleft()
                        emit_av(*psid, pssb)

                # pre-strips: scores for h0 kb0-3 run while V projects, so
                # their exps overlap the V matmuls and AV can start at once
                # (exactly 4: a 5th would emit an AV ahead of V in PE order)
                for sid in strips[:4]:
                    do_strip(sid)

                # V projection in 2-bank sub-phases (kt pairs); hf0 AV only
                # needs kt 0-7, so kt 8-15 interleave into early strips
                vp_pool = tc.tile_pool(name="vp_ps", bufs=2, space="PSUM")
                vp_ctx = vp_pool.__enter__()

                def vp_phase(kt):
                    psv = vp_ctx.tile([P, NH * DH], f32, tag="vp",
                                      name=f"v_{kt}")
                    for ch in range(KCH):
                        nc.tensor.matmul(
                            psv, XT[ch][:, kt * P:(kt + 1) * P],
                            WV[:, ch, :],
                            start=(ch == 0), stop=(ch == KCH - 1))
                    nc.vector.tensor_copy(V[:, kt, :, 0:DH], psv)

                for kt in range(8):
                    vp_phase(kt)
                for si in range(4, 12):
                    do_strip(strips[si])
                    vp_phase(si + 4)
                vp_pool.__exit__(None, None, None)

                # out-proj stream pool opens in the banks vp_ps freed
                op_ps = tc.tile_pool(name="op_ps", bufs=2, space="PSUM")
                op_ctx = op_ps.__enter__()

                # out-proj (qt, dc) units scheduled into the strip stream:
                # q-chunk qc is ready once every head's AV group for qc has
                # been normalized; qc0 triggers at strip 27(+lag), qc1 at 31.
                op_sched = {}
                units01 = [(qt, dc) for qt in range(8) for dc in range(2)]
                for j, u in enumerate(units01):
                    op_sched.setdefault(44 + 3 * j, []).append(u)

                for si in range(12, len(strips)):
                    do_strip(strips[si])
                    for u in op_sched.get(si, ()):
                        emit_opunit(op_ctx, *u, evac=0, dma=nc.sync)
                    if si == 91:
                        # drain so the last head's qc2 AV group closes and
                        # its normalize chain overlaps the final strips
                        while pending:
                            psid, pssb = pending.popleft()
                            emit_av(*psid, pssb)
                while pending:
                    psid, pssb = pending.popleft()
                    emit_av(*psid, pssb)

                # tail: q-chunks 2 and 3 (qc2 ready first), still inside the
                # attention pools (a fresh pool would wait for the full
                # attention-pool close); alternating evac engines keep the
                # 2-bank rotation matmul-bound
                tail = [(qt, dc) for qt in (8, 9, 10, 11) for dc in range(2)]
                tail += [(qt, dc) for qt in (12, 13, 14, 15) for dc in range(2)]
                for j, u in enumerate(tail):
                    emit_opunit(op_ctx, *u, evac=j % 2,
                                dma=nc.sync if j % 2 == 0 else nc.scalar)
                op_ps.__exit__(None, None, None)

    nc.compile()
    return nc


def _get_nc():
    if "nc" not in _CACHE:
        _CACHE["nc"] = _build_nc()
    return _CACHE["nc"]


def _host_inputs(x, W_Q, W_K, W_V, W_O, b_Q, b_K, b_V):
    """Build the 8 per-core input maps (bf16/fp8 pre-cast on host)."""
    bf = ml_dtypes.bfloat16
    e4 = ml_dtypes.float8_e4m3
    x = np.asarray(x, dtype=np.float32)
    scale = 1.0 / np.sqrt(np.float32(DH))
    tri = (np.arange(P)[:, None] <= np.arange(P)[None, :]).astype(bf)

    xts = [np.ascontiguousarray(x[b].T).astype(bf) for b in range(B)]

    def fp8_pack(a):
        # [D, M] -> [128, CP*2*M]: rows p, cols (cp, j, m) where j indexes
        # the two 128-row groups a DoubleRow pass contracts together
        q = np.clip(a, -240.0, 240.0)
        return np.ascontiguousarray(
            q.reshape(CP, 2, P, -1).transpose(2, 0, 1, 3)
            .reshape(P, -1)).astype(e4)

    xqs = [fp8_pack(x[b].T * SX) for b in range(B)] if FP8QK else [None, None]

    def chunked(a):   # [D, M] -> [128, KCH*M] with rows p, cols (ch, m)
        return np.ascontiguousarray(
            a.reshape(KCH, P, -1).transpose(1, 0, 2).reshape(P, -1))

    in_maps = []
    for c in range(NCORES):
        b, hg = divmod(c, NCORES // B)
        h0 = NH * hg
        if FP8QK:
            wq = fp8_pack(np.asarray(W_Q[h0:h0 + NH], np.float32)
                          .reshape(NH * DH, D).T * SW)
            wk = fp8_pack(np.asarray(W_K[h0:h0 + NH], np.float32)
                          .reshape(NH * DH, D).T * SW)
        else:
            wq = chunked((np.asarray(W_Q[h0:h0 + NH], np.float32) * scale)
                         .reshape(NH * DH, D).T).astype(bf)
            wk = chunked(np.asarray(W_K[h0:h0 + NH], np.float32)
                         .reshape(NH * DH, D).T).astype(bf)
        wv = chunked(np.asarray(W_V[h0:h0 + NH], np.float32)
                     .reshape(NH * DH, D).T).astype(bf)
        wo_flat = np.asarray(W_O[h0:h0 + NH], np.float32) \
            .transpose(0, 2, 1).reshape(NH * DH, D)
        wo = np.ascontiguousarray(
            wo_flat.reshape(2, P, D).transpose(1, 0, 2)
            .reshape(P, 2 * D)).astype(bf)
        bq = np.ascontiguousarray(
            (np.asarray(b_Q[h0:h0 + NH], np.float32) * scale)
            .reshape(2, P).T)
        m = {
            "xt": xts[b], "wq": wq, "wk": wk, "wv": wv, "wo": wo,
            "bq": bq, "tri": tri,
        }
        if FP8QK:
            m["xq"] = xqs[b]
        in_maps.append(m)
    return in_maps


def run_spmd(in_maps, **kwargs):
    from concourse import bass_utils
    nc = _get_nc()
    return bass_utils.run_bass_kernel_spmd(
        nc, in_maps, core_ids=list(range(NCORES)), **kwargs)


def kernel(x, W_Q, W_K, W_V, W_O, b_Q, b_K, b_V, b_O):
    in_maps = _host_inputs(x, W_Q, W_K, W_V, W_O, b_Q, b_K, b_V)
    res = run_spmd(in_maps)
    parts = [res.results[c]["out"] for c in range(NCORES)]
    gpb = NCORES // B
    out = np.stack(
        [sum(parts[b * gpb + g] for g in range(gpb)) for b in range(B)], axis=0)
    # b_V shifts every z_h by a constant vector (softmax weights sum to 1),
    # so its whole output contribution is sum_h W_O[h] @ b_V[h]; b_K cancels
    # in the softmax entirely.
    corr = np.einsum("hdk,hk->d", np.asarray(W_O, np.float32),
                     np.asarray(b_V, np.float32))
    out += (np.asarray(b_O, np.float32) + corr)[None, None, :]
    return out.astype(np.float32)


# revision 61
# speedup vs baseline: 1.0010x; 1.0010x over previous
"""Trainium2 Bass kernel for causal multi-head attention (dense transformer).

Problem shapes (hardcoded): x [2,2048,1024], 16 heads x 64 head-dim.
Sharding: data-parallel over batch (2) x tensor-parallel over heads (4/core)
on 8 NeuronCores. Each core computes the partial output (sum over its 4
heads) for one batch element; the host sums the 4 partials per batch and
adds b_O (+ the constant sum_h W_O[h] @ b_V[h] -- b_V shifts every z by a
constant, b_K cancels in softmax, so neither needs device work).

All operands bf16 except the Q/K projections, which run fp8-e4m3 in
DoubleRow perf mode (256-wide contraction per pass, halving projection
matmul time; x scaled by 32 and W_Q/W_K by 1024 into e4m3 range, descaled
in the PSUM evacuation). V / scores / W_O stay bf16 -- fp8 there fails the
accuracy gate. PSUM always accumulates fp32. Other key points:
  - bf16/fp8 host pre-casts halve HBM traffic; no in-flight cast DMAs
  - b_Q is fused into the Q PSUM->SBUF evacuation via ScalarE activation
    bias (per-partition), so no bias matmuls at all
  - QKV projections run chunk-major (contraction-outer) over x^T as it
    streams in; PE warmup matmuls run on a memset tile with no DMA deps
  - scores are S^T[k,q] strips (k on partitions) with the contraction
    zero-padded 64->128 (keeps the PE HAM clock gate warm / 2.4GHz);
    exp fused with PSUM evacuation on ScalarE (bf16 out); causal mask is
    a 0/1 bf16 multiply on the diagonal block (DVE)
  - AV uses V augmented with a ones column so the softmax denominator
    falls out of the same matmul; strips software-pipelined depth 4
  - the output projection is interleaved into the attention strip stream
    (q-chunks 0/1 during the hf=1 strips, 2/3 right after) so the PE
    never idles long enough for the HAM clock gate to throttle and the
    output DMA overlaps compute
"""

import sys

if "/opt/trn_rl_repo" not in sys.path:
    sys.path.insert(0, "/opt/trn_rl_repo")

import numpy as np
import ml_dtypes

B, S, D = 2, 2048, 1024
H, DH = 16, 64
NCORES = 8
NH = 4            # heads per core
KCH = D // 128    # contraction chunks over model dim
CP = D // 256     # fp8 DoubleRow chunk pairs
NT = S // 128     # 128-row tiles over sequence
P = 128

# fp8-e4m3 DoubleRow Q/K projections (scores only; V/W_O stay bf16).
# Host-simulated end-to-end rel err 1.25e-2 vs the 2e-2 gate.
FP8QK = True
SX = 32.0         # x pre-scale into e4m3 range
SW = 1024.0       # W_Q/W_K pre-scale

_CACHE = {}


def _build_nc():
    import concourse.tile as tile
    from concourse import bacc, mybir

    f32 = mybir.dt.float32
    bf16 = mybir.dt.bfloat16
    fp8 = mybir.dt.float8e4
    Exp = mybir.ActivationFunctionType.Exp
    Ident = mybir.ActivationFunctionType.Identity
    mult = mybir.AluOpType.mult
    DR = mybir.MatmulPerfMode.DoubleRow

    nc = bacc.Bacc("TRN2", target_bir_lowering=False, debug=False,
                   num_devices=NCORES)

    xt_d = nc.dram_tensor("xt", [D, S], bf16, kind="ExternalInput").ap()
    if FP8QK:
        xq_d = nc.dram_tensor("xq", [P, CP * S * 2], fp8, kind="ExternalInput").ap()
        wq_d = nc.dram_tensor("wq", [P, CP * NH * DH * 2], fp8, kind="ExternalInput").ap()
        wk_d = nc.dram_tensor("wk", [P, CP * NH * DH * 2], fp8, kind="ExternalInput").ap()
    else:
        wq_d = nc.dram_tensor("wq", [P, KCH * NH * DH], bf16, kind="ExternalInput").ap()
        wk_d = nc.dram_tensor("wk", [P, KCH * NH * DH], bf16, kind="ExternalInput").ap()
    wv_d = nc.dram_tensor("wv", [P, KCH * NH * DH], bf16, kind="ExternalInput").ap()
    wo_d = nc.dram_tensor("wo", [P, 2 * D], bf16, kind="ExternalInput").ap()
    bq_d = nc.dram_tensor("bq", [P, 2], f32, kind="ExternalInput").ap()
    tri_d = nc.dram_tensor("tri", [P, P], bf16, kind="ExternalInput").ap()
    out_d = nc.dram_tensor("out", [S, D], f32, kind="ExternalOutput").ap()

    with tile.TileContext(nc) as tc:
        from contextlib import ExitStack

        with ExitStack() as ctx:
            persist = ctx.enter_context(tc.tile_pool(name="persist", bufs=1))

            QT = persist.tile([P, 2, S], bf16)
            KT = persist.tile([P, NH, S], bf16)
            V = persist.tile([P, NT, NH, DH + 1], bf16)
            ZN = persist.tile([P, 2, S], bf16)
            if FP8QK:
                # DoubleRow operands: [K, cp, j, ·] with j the pair dim
                XQ = persist.tile([P, CP, 2, S], fp8)
                WQ = persist.tile([P, CP, 2, NH * DH], fp8)
                WK = persist.tile([P, CP, 2, NH * DH], fp8)
            else:
                WQ = persist.tile([P, KCH, NH * DH], bf16)
                WK = persist.tile([P, KCH, NH * DH], bf16)
            WV = persist.tile([P, KCH, NH * DH], bf16)
            WO = persist.tile([P, 2, D], bf16)
            BQ = persist.tile([P, 2], f32)
            TRI = persist.tile([P, P], bf16)
            WRM = persist.tile([P, P], bf16)

            # ---- t0: memsets (no DMA deps) + input DMA kickoff; Q/K
            # weights lead the two fast HWDGE rings (the gpsimd SWDGE ring
            # is slow to spin up and round-robins across its queued DMAs) ----
            nc.vector.memset(WRM, 0.0)
            nc.vector.memset(KT, 0.0)
            nc.vector.memset(V[:, :, :, DH:DH + 1], 1.0)

            nc.gpsimd.dma_start(BQ, bq_d)
            nc.gpsimd.dma_start(TRI, tri_d)
            nc.gpsimd.dma_start(WV.rearrange("p a b -> p (a b)"), wv_d)
            nc.gpsimd.dma_start(WO.rearrange("p a b -> p (a b)"), wo_d)

            xt_ctx = ctx.enter_context(tc.tile_pool(name="xt", bufs=1))
            XT = [xt_ctx.tile([P, S], bf16, name=f"xt{ch}")
                  for ch in range(KCH)]

            if FP8QK:
                nc.sync.dma_start(WK.rearrange("p a b c -> p (a b c)"), wk_d)
                nc.scalar.dma_start(WQ.rearrange("p a b c -> p (a b c)"), wq_d)
                for cp in range(CP):
                    eng = nc.sync if cp % 2 == 0 else nc.scalar
                    eng.dma_start(
                        XQ[:, cp, :, :],
                        xq_d[:, cp * 2 * S:(cp + 1) * 2 * S])
            else:
                wkr = WK.rearrange("p a b -> p (a b)")
                wqr = WQ.rearrange("p a b -> p (a b)")
                hw = KCH * NH * DH // 2
                nc.sync.dma_start(wkr[:, 0:hw], wk_d[:, 0:hw])
                nc.scalar.dma_start(wqr[:, 0:hw], wq_d[:, 0:hw])
                nc.sync.dma_start(wkr[:, hw:], wk_d[:, hw:])
                nc.scalar.dma_start(wqr[:, hw:], wq_d[:, hw:])
            for ch in range(KCH):
                eng = nc.sync if ch % 2 == 0 else nc.scalar
                eng.dma_start(XT[ch], xt_d[ch * P:(ch + 1) * P, :])

            # ---- PE warmup while input DMAs stream (warms HAM clock) ----
            with tc.tile_pool(name="warm_ps", bufs=1, space="PSUM") as wp:
                wps = wp.tile([P, P], f32)
                for _ in range(24):
                    nc.tensor.matmul(wps, WRM, WRM, start=True, stop=True)

            # ---- phase 1: K projection sweep, then Q sweep (K first so its
            # 16 split-plane evacuations overlap the Q matmuls; Q's 8
            # bias-evacs split across ScalarE/DVE are the only bank-release
            # latency the attention pools then wait on) ----
            add = mybir.AluOpType.add
            dsq = 1.0 / (SX * SW * 8.0) if FP8QK else 1.0   # Q descale (attn
            dsk = 1.0 / (SX * SW) if FP8QK else 1.0         # scale on Q side)
            with tc.tile_pool(name="qk_ps", bufs=8, space="PSUM") as qk_ps:
                for wi, W_ in ((1, WK), (0, WQ)):
                    pst = {}
                    for t in range(2):
                        for qc in range(4):
                            pst[(t, qc)] = qk_ps.tile(
                                [P, 512], f32, tag="qk",
                                name=f"qk{wi}_{t}_{qc}")
                    if FP8QK:
                        for cp in range(CP):
                            for t in range(2):
                                for qc in range(4):
                                    nc.tensor.matmul(
                                        pst[(t, qc)],
                                        W_[:, cp, :, t * P:(t + 1) * P],
                                        XQ[:, cp, :, qc * 512:(qc + 1) * 512],
                                        start=(cp == 0), stop=(cp == CP - 1),
                                        perf_mode=DR)
                    else:
                        for ch in range(KCH):
                            for t in range(2):
                                for qc in range(4):
                                    nc.tensor.matmul(
                                        pst[(t, qc)],
                                        W_[:, ch, t * P:(t + 1) * P],
                                        XT[ch][:, qc * 512:(qc + 1) * 512],
                                        start=(ch == 0), stop=(ch == KCH - 1))
                    for t in range(2):
                        for qc in range(4):
                            sl = slice(qc * 512, (qc + 1) * 512)
                            ps = pst[(t, qc)]
                            if wi == 1:
                                # K: split head halves into zero-padded planes
                                if FP8QK:
                                    nc.scalar.mul(
                                        KT[0:64, 2 * t, sl], ps[0:64, :], dsk)
                                    nc.vector.tensor_scalar_mul(
                                        KT[64:128, 2 * t + 1, sl],
                                        ps[64:128, :], dsk)
                                else:
                                    nc.scalar.copy(
                                        KT[0:64, 2 * t, sl], ps[0:64, :])
                                    nc.vector.tensor_copy(
                                        KT[64:128, 2 * t + 1, sl],
                                        ps[64:128, :])
                            elif t == 0:
                                # Q: evacuate with b_Q fused as bias
                                nc.scalar.activation(
                                    QT[:, t, sl], ps, Ident,
                                    bias=BQ[:, t:t + 1], scale=dsq)
                            elif FP8QK:
                                nc.vector.tensor_scalar(
                                    QT[:, t, sl], ps, dsq, BQ[:, t:t + 1],
                                    mult, add)
                            else:
                                nc.vector.tensor_scalar(
                                    QT[:, t, sl], ps, BQ[:, t:t + 1],
                                    None, add)

            # ---- phase 2: attention strips, V projection folded into the
            # pipeline fill, out-projection interleaved into the stream ----
            with tc.tile_pool(name="esp", bufs=6) as esp, \
                    tc.tile_pool(name="nrm", bufs=4) as nrm, \
                    tc.tile_pool(name="osb", bufs=4) as osb, \
                    tc.tile_pool(name="sc_ps", bufs=2, space="PSUM") as sc_ps, \
                    tc.tile_pool(name="av_ps", bufs=2, space="PSUM") as av_ps:
                avs = {}

                def emit_scores(h, kb, hf):
                    t = h // 2
                    k0 = kb * P
                    hstart = hf * 1024
                    qstart = max(k0, hstart)
                    sps = sc_ps.tile([P, 1024], f32,
                                     name=f"sps_{h}_{kb}_{hf}", tag="sps")
                    ssb = esp.tile([P, 1024], bf16,
                                   name=f"ssb_{h}_{kb}_{hf}", tag="ssb")
                    qpos = qstart
                    while qpos < hstart + 1024:
                        qnext = min(hstart + 1024, (qpos // 512 + 1) * 512)
                        nc.tensor.matmul(
                            sps[:, qpos - hstart:qnext - hstart],
                            KT[:, h, k0:k0 + P],
                            QT[:, t, qpos:qnext],
                            start=True, stop=True)
                        qpos = qnext
                    nc.scalar.activation(
                        ssb[:, qstart - hstart:1024],
                        sps[:, qstart - hstart:1024], Exp)
                    if k0 >= hstart:
                        dsl = slice(k0 - hstart, k0 - hstart + P)
                        nc.vector.tensor_tensor(
                            ssb[:, dsl], ssb[:, dsl], TRI, mult)
                    return ssb

                def emit_norm(h, qc, avq):
                    t, pb = h // 2, (h % 2) * 64
                    rd = nrm.tile([1, 512], f32, tag="rd")
                    nc.vector.tensor_copy(rd, avq[DH:DH + 1, :])
                    rr = nrm.tile([1, 512], f32, tag="rr")
                    nc.vector.reciprocal_approx_fast(out=rr, in_=rd)
                    rdb = nrm.tile([64, 512], f32, tag="rdb")
                    nc.gpsimd.partition_broadcast(rdb, rr)
                    zslc = ZN[pb:pb + 64, t, qc * 512:(qc + 1) * 512]
                    nc.vector.tensor_tensor(zslc, avq[0:DH, :], rdb, mult)

                def emit_av(h, kb, hf, ssb):
                    k0 = kb * P
                    hstart = hf * 1024
                    qstart = max(k0, hstart)
                    if kb == 0:
                        for qc in (2 * hf, 2 * hf + 1):
                            avs[(h, qc)] = av_ps.tile(
                                [DH + 1, 512], f32,
                                tag="av", name=f"av_{h}_{qc}")
                    qpos = qstart
                    while qpos < hstart + 1024:
                        qc = qpos // 512
                        qnext = min(hstart + 1024, (qc + 1) * 512)
                        done = kb == 4 * qc + 3
                        nc.tensor.matmul(
                            avs[(h, qc)][:, qpos - qc * 512:qnext - qc * 512],
                            V[:, kb, h, :],
                            ssb[:, qpos - hstart:qnext - hstart],
                            start=(kb == 0), stop=done)
                        if done:
                            emit_norm(h, qc, avs[(h, qc)])
                        qpos = qnext

                def emit_opunit(pool, qt, dc, evac, dma):
                    ps = pool.tile([P, 512], f32, tag="op",
                                   name=f"op_{qt}_{dc}")
                    for t in range(2):
                        nc.tensor.matmul(
                            ps, ZN[:, t, qt * P:(qt + 1) * P],
                            WO[:, t, dc * 512:(dc + 1) * 512],
                            start=(t == 0), stop=(t == 1))
                    ob = osb.tile([P, 512], f32, tag="osb",
                                  name=f"ob_{qt}_{dc}")
                    if evac == 0:
                        nc.vector.tensor_copy(ob, ps)
                    else:
                        nc.scalar.copy(ob, ps)
                    dma.dma_start(
                        out_d[qt * P:(qt + 1) * P, dc * 512:(dc + 1) * 512],
                        ob)

                strips = [(h, kb, 0) for h in range(NH) for kb in range(8)]
                strips += [(h, kb, 1) for h in range(NH) for kb in range(NT)]

                from collections import deque
                pending = deque()

                def do_strip(sid):
                    ssb = emit_scores(*sid)
                    pending.append((sid, ssb))
                    if len(pending) > 4:
                        psid, pssb = pending.popleft()
                        emit_av(*psid, pssb)

                # pre-strips: scores for h0 kb0-3 run while V projects, so
                # their exps overlap the V matmuls and AV can start at once
                # (exactly 4: a 5th would emit an AV ahead of V in PE order)
                for sid in strips[:4]:
                    do_strip(sid)

                # V projection in 2-bank sub-phases (kt pairs); hf0 AV only
                # needs kt 0-7, so kt 8-15 interleave into early strips
                vp_pool = tc.tile_pool(name="vp_ps", bufs=2, space="PSUM")
                vp_ctx = vp_pool.__enter__()

                def vp_phase(kt):
                    psv = vp_ctx.tile([P, NH * DH], f32, tag="vp",
                                      name=f"v_{kt}")
                    for ch in range(KCH):
                        nc.tensor.matmul(
                            psv, XT[ch][:, kt * P:(kt + 1) * P],
                            WV[:, ch, :],
                            start=(ch == 0), stop=(ch == KCH - 1))
                    nc.vector.tensor_copy(V[:, kt, :, 0:DH], psv)

                for kt in range(8):
                    vp_phase(kt)
                for si in range(4, 12):
                    do_strip(strips[si])
                    vp_phase(si + 4)
                vp_pool.__exit__(None, None, None)

                # out-proj stream pool opens in the banks vp_ps freed
                op_ps = tc.tile_pool(name="op_ps", bufs=2, space="PSUM")
                op_ctx = op_ps.__enter__()

                # out-proj (qt, dc) units scheduled into the strip stream:
                # q-chunk qc is ready once every head's AV group for qc has
                # been normalized; qc0 triggers at strip 27(+lag), qc1 at 31.
                op_sched = {}
                units01 = [(qt, dc) for qt in range(8) for dc in range(2)]
                for j, u in enumerate(units01):
                    op_sched.setdefault(44 + 3 * j, []).append(u)

                for si in range(12, len(strips)):
                    do_strip(strips[si])
                    for u in op_sched.get(si, ()):
                        emit_opunit(op_ctx, *u, evac=0, dma=nc.sync)
                    if si == 91:
                        # drain so the last head's qc2 AV group closes and
                        # its normalize chain overlaps the final strips
                        while pending:
                            psid, pssb = pending.popleft()
                            emit_av(*psid, pssb)
                while pending:
                    psid, pssb = pending.popleft()
                    emit_av(*psid, pssb)

                # tail: q-chunks 2 and 3 (qc2 ready first), still inside the
                # attention pools (a fresh pool would wait for the full
                # attention-pool close); alternating evac engines keep the
                # 2-bank rotation matmul-bound
                tail = [(qt, dc) for qt in (8, 9, 10, 11) for dc in range(2)]
                tail += [(qt, dc) for qt in (12, 13, 14, 15) for dc in range(2)]
                for j, u in enumerate(tail):
                    emit_opunit(op_ctx, *u, evac=j % 2,
                                dma=nc.sync if j % 2 == 0 else nc.scalar)
                op_ps.__exit__(None, None, None)

    nc.compile()
    return nc


def _get_nc():
    if "nc" not in _CACHE:
        _CACHE["nc"] = _build_nc()
    return _CACHE["nc"]


def _host_inputs(x, W_Q, W_K, W_V, W_O, b_Q, b_K, b_V):
    """Build the 8 per-core input maps (bf16/fp8 pre-cast on host)."""
    bf = ml_dtypes.bfloat16
    e4 = ml_dtypes.float8_e4m3
    x = np.asarray(x, dtype=np.float32)
    scale = 1.0 / np.sqrt(np.float32(DH))
    tri = (np.arange(P)[:, None] <= np.arange(P)[None, :]).astype(bf)

    xts = [np.ascontiguousarray(x[b].T).astype(bf) for b in range(B)]

    def fp8_pack(a):
        # [D, M] -> [128, CP*2*M]: rows p, cols (cp, j, m) where j indexes
        # the two 128-row groups a DoubleRow pass contracts together
        q = np.clip(a, -240.0, 240.0)
        return np.ascontiguousarray(
            q.reshape(CP, 2, P, -1).transpose(2, 0, 1, 3)
            .reshape(P, -1)).astype(e4)

    xqs = [fp8_pack(x[b].T * SX) for b in range(B)] if FP8QK else [None, None]

    def chunked(a):   # [D, M] -> [128, KCH*M] with rows p, cols (ch, m)
        return np.ascontiguousarray(
            a.reshape(KCH, P, -1).transpose(1, 0, 2).reshape(P, -1))

    in_maps = []
    for c in range(NCORES):
        b, hg = divmod(c, NCORES // B)
        h0 = NH * hg
        if FP8QK:
            wq = fp8_pack(np.asarray(W_Q[h0:h0 + NH], np.float32)
                          .reshape(NH * DH, D).T * SW)
            wk = fp8_pack(np.asarray(W_K[h0:h0 + NH], np.float32)
                          .reshape(NH * DH, D).T * SW)
        else:
            wq = chunked((np.asarray(W_Q[h0:h0 + NH], np.float32) * scale)
                         .reshape(NH * DH, D).T).astype(bf)
            wk = chunked(np.asarray(W_K[h0:h0 + NH], np.float32)
                         .reshape(NH * DH, D).T).astype(bf)
        wv = chunked(np.asarray(W_V[h0:h0 + NH], np.float32)
                     .reshape(NH * DH, D).T).astype(bf)
        wo_flat = np.asarray(W_O[h0:h0 + NH], np.float32) \
            .transpose(0, 2, 1).reshape(NH * DH, D)
        wo = np.ascontiguousarray(
            wo_flat.reshape(2, P, D).transpose(1, 0, 2)
            .reshape(P, 2 * D)).astype(bf)
        bq = np.ascontiguousarray(
            (np.asarray(b_Q[h0:h0 + NH], np.float32) * scale)
            .reshape(2, P).T)
        m = {
            "xt": xts[b], "wq": wq, "wk": wk, "wv": wv, "wo": wo,
            "bq": bq, "tri": tri,
        }
        if FP8QK:
            m["xq"] = xqs[b]
        in_maps.append(m)
    return in_maps


def run_spmd(in_maps, **kwargs):
    from concourse import bass_utils
    nc = _get_nc()
    return bass_utils.run_bass_kernel_spmd(
        nc, in_maps, core_ids=list(range(NCORES)), **kwargs)


def kernel(x, W_Q, W_K, W_V, W_O, b_Q, b_K, b_V, b_O):
    in_maps = _host_inputs(x, W_Q, W_K, W_V, W_O, b_Q, b_K, b_V)
    res = run_spmd(in_maps)
    parts = [res.results[c]["out"] for c in range(NCORES)]
    gpb = NCORES // B
    out = np.stack(
        [sum(parts[b * gpb + g] for g in range(gpb)) for b in range(B)], axis=0)
    # b_V shifts every z_h by a constant vector (softmax weights sum to 1),
    # so its whole output contribution is sum_h W_O[h] @ b_V[h]; b_K cancels
    # in the softmax entirely.
    corr = np.einsum("hdk,hk->d", np.asarray(W_O, np.float32),
                     np.asarray(b_V, np.float32))
    out += (np.asarray(b_O, np.float32) + corr)[None, None, :]
    return out.astype(np.float32)


# revision 62
# speedup vs baseline: 1.0109x; 1.0099x over previous
"""Trainium2 Bass kernel for causal multi-head attention (dense transformer).

Problem shapes (hardcoded): x [2,2048,1024], 16 heads x 64 head-dim.
Sharding: data-parallel over batch (2) x tensor-parallel over heads (4/core)
on 8 NeuronCores. Each core computes the partial output (sum over its 4
heads) for one batch element; the host sums the 4 partials per batch and
adds b_O (+ the constant sum_h W_O[h] @ b_V[h] -- b_V shifts every z by a
constant, b_K cancels in softmax, so neither needs device work).

All operands bf16 except the Q/K projections, which run fp8-e4m3 in
DoubleRow perf mode (256-wide contraction per pass, halving projection
matmul time; x scaled by 32 and W_Q/W_K by 1024 into e4m3 range, descaled
in the PSUM evacuation). V / scores / W_O stay bf16 -- fp8 there fails the
accuracy gate. PSUM always accumulates fp32. Other key points:
  - bf16/fp8 host pre-casts halve HBM traffic; no in-flight cast DMAs
  - b_Q is fused into the Q PSUM->SBUF evacuation via ScalarE activation
    bias (per-partition), so no bias matmuls at all
  - QKV projections run chunk-major (contraction-outer) over x^T as it
    streams in; PE warmup matmuls run on a memset tile with no DMA deps
  - scores are S^T[k,q] strips (k on partitions) with the contraction
    zero-padded 64->128 (keeps the PE HAM clock gate warm / 2.4GHz);
    exp fused with PSUM evacuation on ScalarE (bf16 out); causal mask is
    a 0/1 bf16 multiply on the diagonal block (DVE)
  - AV uses V augmented with a ones column so the softmax denominator
    falls out of the same matmul; strips software-pipelined depth 4
  - the output projection is interleaved into the attention strip stream
    (q-chunks 0/1 during the hf=1 strips, 2/3 right after) so the PE
    never idles long enough for the HAM clock gate to throttle and the
    output DMA overlaps compute
"""

import sys

if "/opt/trn_rl_repo" not in sys.path:
    sys.path.insert(0, "/opt/trn_rl_repo")

import numpy as np
import ml_dtypes

B, S, D = 2, 2048, 1024
H, DH = 16, 64
NCORES = 8
NH = 4            # heads per core
KCH = D // 128    # contraction chunks over model dim
CP = D // 256     # fp8 DoubleRow chunk pairs
NT = S // 128     # 128-row tiles over sequence
P = 128

# fp8-e4m3 DoubleRow Q/K projections (scores only; V/W_O stay bf16).
# Host-simulated end-to-end rel err 1.25e-2 vs the 2e-2 gate.
FP8QK = True
SX = 32.0         # x pre-scale into e4m3 range
SW = 1024.0       # W_Q/W_K pre-scale

_CACHE = {}


def _build_nc():
    import concourse.tile as tile
    from concourse import bacc, mybir

    f32 = mybir.dt.float32
    bf16 = mybir.dt.bfloat16
    fp8 = mybir.dt.float8e4
    Exp = mybir.ActivationFunctionType.Exp
    Ident = mybir.ActivationFunctionType.Identity
    mult = mybir.AluOpType.mult
    DR = mybir.MatmulPerfMode.DoubleRow

    nc = bacc.Bacc("TRN2", target_bir_lowering=False, debug=False,
                   num_devices=NCORES)

    xt_d = nc.dram_tensor("xt", [D, S], bf16, kind="ExternalInput").ap()
    if FP8QK:
        xq_d = nc.dram_tensor("xq", [P, CP * S * 2], fp8, kind="ExternalInput").ap()
        wq_d = nc.dram_tensor("wq", [P, CP * NH * DH * 2], fp8, kind="ExternalInput").ap()
        wk_d = nc.dram_tensor("wk", [P, CP * NH * DH * 2], fp8, kind="ExternalInput").ap()
    else:
        wq_d = nc.dram_tensor("wq", [P, KCH * NH * DH], bf16, kind="ExternalInput").ap()
        wk_d = nc.dram_tensor("wk", [P, KCH * NH * DH], bf16, kind="ExternalInput").ap()
    wv_d = nc.dram_tensor("wv", [P, KCH * NH * DH], bf16, kind="ExternalInput").ap()
    wo_d = nc.dram_tensor("wo", [P, 2 * D], bf16, kind="ExternalInput").ap()
    bq_d = nc.dram_tensor("bq", [P, 2], f32, kind="ExternalInput").ap()
    tri_d = nc.dram_tensor("tri", [P, P], bf16, kind="ExternalInput").ap()
    out_d = nc.dram_tensor("out", [S, D], f32, kind="ExternalOutput").ap()

    with tile.TileContext(nc) as tc:
        from contextlib import ExitStack

        with ExitStack() as ctx:
            persist = ctx.enter_context(tc.tile_pool(name="persist", bufs=1))

            QT = persist.tile([P, 2, S], bf16)
            KT = persist.tile([P, NH, S], bf16)
            V = persist.tile([P, NT, NH, DH + 1], bf16)
            ZN = persist.tile([P, 2, S], bf16)
            if FP8QK:
                # DoubleRow operands: [K, cp, j, ·] with j the pair dim
                XQ = persist.tile([P, CP, 2, S], fp8)
                WQ = persist.tile([P, CP, 2, NH * DH], fp8)
                WK = persist.tile([P, CP, 2, NH * DH], fp8)
            else:
                WQ = persist.tile([P, KCH, NH * DH], bf16)
                WK = persist.tile([P, KCH, NH * DH], bf16)
            WV = persist.tile([P, KCH, NH * DH], bf16)
            WO = persist.tile([P, 2, D], bf16)
            BQ = persist.tile([P, 2], f32)
            TRI = persist.tile([P, P], bf16)
            WRM = persist.tile([P, P], bf16)

            # ---- t0: memsets (no DMA deps) + input DMA kickoff; Q/K
            # weights lead the two fast HWDGE rings (the gpsimd SWDGE ring
            # is slow to spin up and round-robins across its queued DMAs) ----
            nc.vector.memset(WRM, 0.0)
            nc.vector.memset(KT, 0.0)
            nc.vector.memset(V[:, :, :, DH:DH + 1], 1.0)

            nc.gpsimd.dma_start(BQ, bq_d)
            nc.gpsimd.dma_start(TRI, tri_d)
            nc.gpsimd.dma_start(WV.rearrange("p a b -> p (a b)"), wv_d)
            nc.gpsimd.dma_start(WO.rearrange("p a b -> p (a b)"), wo_d)

            xt_ctx = ctx.enter_context(tc.tile_pool(name="xt", bufs=1))
            XT = [xt_ctx.tile([P, S], bf16, name=f"xt{ch}")
                  for ch in range(KCH)]

            if FP8QK:
                nc.sync.dma_start(WK.rearrange("p a b c -> p (a b c)"), wk_d)
                nc.scalar.dma_start(WQ.rearrange("p a b c -> p (a b c)"), wq_d)
                for cp in range(CP):
                    eng = nc.sync if cp % 2 == 0 else nc.scalar
                    eng.dma_start(
                        XQ[:, cp, :, :],
                        xq_d[:, cp * 2 * S:(cp + 1) * 2 * S])
            else:
                wkr = WK.rearrange("p a b -> p (a b)")
                wqr = WQ.rearrange("p a b -> p (a b)")
                hw = KCH * NH * DH // 2
                nc.sync.dma_start(wkr[:, 0:hw], wk_d[:, 0:hw])
                nc.scalar.dma_start(wqr[:, 0:hw], wq_d[:, 0:hw])
                nc.sync.dma_start(wkr[:, hw:], wk_d[:, hw:])
                nc.scalar.dma_start(wqr[:, hw:], wq_d[:, hw:])
            for ch in range(KCH):
                eng = nc.sync if ch % 2 == 0 else nc.scalar
                eng.dma_start(XT[ch], xt_d[ch * P:(ch + 1) * P, :])

            # ---- PE warmup while input DMAs stream: 48 matmuls bridge the
            # DMA wait so the HAM clock gate doesn't re-throttle and the
            # first sweep matmuls start at 2.4GHz (measured: 24 left a 4us
            # idle window and the first 8 sweep MMs ran at 1.2GHz) ----
            with tc.tile_pool(name="warm_ps", bufs=1, space="PSUM") as wp:
                wps = wp.tile([P, P], f32)
                for _ in range(48):
                    nc.tensor.matmul(wps, WRM, WRM, start=True, stop=True)

            # ---- phase 1: K projection sweep, then Q sweep (K first so its
            # 16 split-plane evacuations overlap the Q matmuls; Q's 8
            # bias-evacs split across ScalarE/DVE are the only bank-release
            # latency the attention pools then wait on) ----
            add = mybir.AluOpType.add
            dsq = 1.0 / (SX * SW * 8.0) if FP8QK else 1.0   # Q descale (attn
            dsk = 1.0 / (SX * SW) if FP8QK else 1.0         # scale on Q side)
            with tc.tile_pool(name="qk_ps", bufs=8, space="PSUM") as qk_ps:
                for wi, W_ in ((1, WK), (0, WQ)):
                    pst = {}
                    for t in range(2):
                        for qc in range(4):
                            pst[(t, qc)] = qk_ps.tile(
                                [P, 512], f32, tag="qk",
                                name=f"qk{wi}_{t}_{qc}")
                    if FP8QK:
                        for cp in range(CP):
                            for t in range(2):
                                for qc in range(4):
                                    nc.tensor.matmul(
                                        pst[(t, qc)],
                                        W_[:, cp, :, t * P:(t + 1) * P],
                                        XQ[:, cp, :, qc * 512:(qc + 1) * 512],
                                        start=(cp == 0), stop=(cp == CP - 1),
                                        perf_mode=DR)
                    else:
                        for ch in range(KCH):
                            for t in range(2):
                                for qc in range(4):
                                    nc.tensor.matmul(
                                        pst[(t, qc)],
                                        W_[:, ch, t * P:(t + 1) * P],
                                        XT[ch][:, qc * 512:(qc + 1) * 512],
                                        start=(ch == 0), stop=(ch == KCH - 1))
                    for t in range(2):
                        for qc in range(4):
                            sl = slice(qc * 512, (qc + 1) * 512)
                            ps = pst[(t, qc)]
                            if wi == 1:
                                # K: split head halves into zero-padded planes
                                if FP8QK:
                                    nc.scalar.mul(
                                        KT[0:64, 2 * t, sl], ps[0:64, :], dsk)
                                    nc.vector.tensor_scalar_mul(
                                        KT[64:128, 2 * t + 1, sl],
                                        ps[64:128, :], dsk)
                                else:
                                    nc.scalar.copy(
                                        KT[0:64, 2 * t, sl], ps[0:64, :])
                                    nc.vector.tensor_copy(
                                        KT[64:128, 2 * t + 1, sl],
                                        ps[64:128, :])
                            elif t == 0:
                                # Q: evacuate with b_Q fused as bias
                                nc.scalar.activation(
                                    QT[:, t, sl], ps, Ident,
                                    bias=BQ[:, t:t + 1], scale=dsq)
                            elif FP8QK:
                                nc.vector.tensor_scalar(
                                    QT[:, t, sl], ps, dsq, BQ[:, t:t + 1],
                                    mult, add)
                            else:
                                nc.vector.tensor_scalar(
                                    QT[:, t, sl], ps, BQ[:, t:t + 1],
                                    None, add)

            # ---- phase 2: attention strips, V projection folded into the
            # pipeline fill, out-projection interleaved into the stream ----
            with tc.tile_pool(name="esp", bufs=6) as esp, \
                    tc.tile_pool(name="nrm", bufs=4) as nrm, \
                    tc.tile_pool(name="osb", bufs=4) as osb, \
                    tc.tile_pool(name="sc_ps", bufs=2, space="PSUM") as sc_ps, \
                    tc.tile_pool(name="av_ps", bufs=2, space="PSUM") as av_ps:
                avs = {}

                def emit_scores(h, kb, hf):
                    t = h // 2
                    k0 = kb * P
                    hstart = hf * 1024
                    qstart = max(k0, hstart)
                    sps = sc_ps.tile([P, 1024], f32,
                                     name=f"sps_{h}_{kb}_{hf}", tag="sps")
                    ssb = esp.tile([P, 1024], bf16,
                                   name=f"ssb_{h}_{kb}_{hf}", tag="ssb")
                    qpos = qstart
                    while qpos < hstart + 1024:
                        qnext = min(hstart + 1024, (qpos // 512 + 1) * 512)
                        nc.tensor.matmul(
                            sps[:, qpos - hstart:qnext - hstart],
                            KT[:, h, k0:k0 + P],
                            QT[:, t, qpos:qnext],
                            start=True, stop=True)
                        qpos = qnext
                    nc.scalar.activation(
                        ssb[:, qstart - hstart:1024],
                        sps[:, qstart - hstart:1024], Exp)
                    if k0 >= hstart:
                        dsl = slice(k0 - hstart, k0 - hstart + P)
                        nc.vector.tensor_tensor(
                            ssb[:, dsl], ssb[:, dsl], TRI, mult)
                    return ssb

                def emit_norm(h, qc, avq):
                    t, pb = h // 2, (h % 2) * 64
                    rd = nrm.tile([1, 512], f32, tag="rd")
                    nc.vector.tensor_copy(rd, avq[DH:DH + 1, :])
                    rr = nrm.tile([1, 512], f32, tag="rr")
                    nc.vector.reciprocal_approx_fast(out=rr, in_=rd)
                    rdb = nrm.tile([64, 512], f32, tag="rdb")
                    nc.gpsimd.partition_broadcast(rdb, rr)
                    zslc = ZN[pb:pb + 64, t, qc * 512:(qc + 1) * 512]
                    nc.vector.tensor_tensor(zslc, avq[0:DH, :], rdb, mult)

                def emit_av(h, kb, hf, ssb):
                    k0 = kb * P
                    hstart = hf * 1024
                    qstart = max(k0, hstart)
                    if kb == 0:
                        for qc in (2 * hf, 2 * hf + 1):
                            avs[(h, qc)] = av_ps.tile(
                                [DH + 1, 512], f32,
                                tag="av", name=f"av_{h}_{qc}")
                    qpos = qstart
                    while qpos < hstart + 1024:
                        qc = qpos // 512
                        qnext = min(hstart + 1024, (qc + 1) * 512)
                        done = kb == 4 * qc + 3
                        nc.tensor.matmul(
                            avs[(h, qc)][:, qpos - qc * 512:qnext - qc * 512],
                            V[:, kb, h, :],
                            ssb[:, qpos - hstart:qnext - hstart],
                            start=(kb == 0), stop=done)
                        if done:
                            emit_norm(h, qc, avs[(h, qc)])
                        qpos = qnext

                def emit_opunit(pool, qt, dc, evac, dma):
                    ps = pool.tile([P, 512], f32, tag="op",
                                   name=f"op_{qt}_{dc}")
                    for t in range(2):
                        nc.tensor.matmul(
                            ps, ZN[:, t, qt * P:(qt + 1) * P],
                            WO[:, t, dc * 512:(dc + 1) * 512],
                            start=(t == 0), stop=(t == 1))
                    ob = osb.tile([P, 512], f32, tag="osb",
                                  name=f"ob_{qt}_{dc}")
                    if evac == 0:
                        nc.vector.tensor_copy(ob, ps)
                    else:
                        nc.scalar.copy(ob, ps)
                    dma.dma_start(
                        out_d[qt * P:(qt + 1) * P, dc * 512:(dc + 1) * 512],
                        ob)

                strips = [(h, kb, 0) for h in range(NH) for kb in range(8)]
                strips += [(h, kb, 1) for h in range(NH) for kb in range(NT)]

                from collections import deque
                pending = deque()

                def do_strip(sid):
                    ssb = emit_scores(*sid)
                    pending.append((sid, ssb))
                    if len(pending) > 4:
                        psid, pssb = pending.popleft()
                        emit_av(*psid, pssb)

                # pre-strips: scores for h0 kb0-3 run while V projects, so
                # their exps overlap the V matmuls and AV can start at once
                # (exactly 4: a 5th would emit an AV ahead of V in PE order)
                for sid in strips[:4]:
                    do_strip(sid)

                # V projection in 2-bank sub-phases (kt pairs); hf0 AV only
                # needs kt 0-7, so kt 8-15 interleave into early strips
                vp_pool = tc.tile_pool(name="vp_ps", bufs=2, space="PSUM")
                vp_ctx = vp_pool.__enter__()

                def vp_phase(kt):
                    psv = vp_ctx.tile([P, NH * DH], f32, tag="vp",
                                      name=f"v_{kt}")
                    for ch in range(KCH):
                        nc.tensor.matmul(
                            psv, XT[ch][:, kt * P:(kt + 1) * P],
                            WV[:, ch, :],
                            start=(ch == 0), stop=(ch == KCH - 1))
                    nc.vector.tensor_copy(V[:, kt, :, 0:DH], psv)

                for kt in range(8):
                    vp_phase(kt)
                for si in range(4, 12):
                    do_strip(strips[si])
                    vp_phase(si + 4)
                vp_pool.__exit__(None, None, None)

                # out-proj stream pool opens in the banks vp_ps freed
                op_ps = tc.tile_pool(name="op_ps", bufs=2, space="PSUM")
                op_ctx = op_ps.__enter__()

                # out-proj (qt, dc) units scheduled into the strip stream:
                # q-chunk qc is ready once every head's AV group for qc has
                # been normalized; qc0 triggers at strip 27(+lag), qc1 at 31.
                op_sched = {}
                units01 = [(qt, dc) for qt in range(8) for dc in range(2)]
                for j, u in enumerate(units01):
                    op_sched.setdefault(44 + 3 * j, []).append(u)

                for si in range(12, len(strips)):
                    do_strip(strips[si])
                    for u in op_sched.get(si, ()):
                        emit_opunit(op_ctx, *u, evac=0, dma=nc.sync)
                    if si == 91:
                        # drain so the last head's qc2 AV group closes and
                        # its normalize chain overlaps the final strips
                        while pending:
                            psid, pssb = pending.popleft()
                            emit_av(*psid, pssb)
                while pending:
                    psid, pssb = pending.popleft()
                    emit_av(*psid, pssb)

                # tail: q-chunks 2 and 3 (qc2 ready first), still inside the
                # attention pools (a fresh pool would wait for the full
                # attention-pool close); alternating evac engines keep the
                # 2-bank rotation matmul-bound
                tail = [(qt, dc) for qt in (8, 9, 10, 11) for dc in range(2)]
                tail += [(qt, dc) for qt in (12, 13, 14, 15) for dc in range(2)]
                for j, u in enumerate(tail):
                    emit_opunit(op_ctx, *u, evac=j % 2,
                                dma=nc.sync if j % 2 == 0 else nc.scalar)
                op_ps.__exit__(None, None, None)

    nc.compile()
    return nc


def _get_nc():
    if "nc" not in _CACHE:
        _CACHE["nc"] = _build_nc()
    return _CACHE["nc"]


def _host_inputs(x, W_Q, W_K, W_V, W_O, b_Q, b_K, b_V):
    """Build the 8 per-core input maps (bf16/fp8 pre-cast on host)."""
    bf = ml_dtypes.bfloat16
    e4 = ml_dtypes.float8_e4m3
    x = np.asarray(x, dtype=np.float32)
    scale = 1.0 / np.sqrt(np.float32(DH))
    tri = (np.arange(P)[:, None] <= np.arange(P)[None, :]).astype(bf)

    xts = [np.ascontiguousarray(x[b].T).astype(bf) for b in range(B)]

    def fp8_pack(a):
        # [D, M] -> [128, CP*2*M]: rows p, cols (cp, j, m) where j indexes
        # the two 128-row groups a DoubleRow pass contracts together
        q = np.clip(a, -240.0, 240.0)
        return np.ascontiguousarray(
            q.reshape(CP, 2, P, -1).transpose(2, 0, 1, 3)
            .reshape(P, -1)).astype(e4)

    xqs = [fp8_pack(x[b].T * SX) for b in range(B)] if FP8QK else [None, None]

    def chunked(a):   # [D, M] -> [128, KCH*M] with rows p, cols (ch, m)
        return np.ascontiguousarray(
            a.reshape(KCH, P, -1).transpose(1, 0, 2).reshape(P, -1))

    in_maps = []
    for c in range(NCORES):
        b, hg = divmod(c, NCORES // B)
        h0 = NH * hg
        if FP8QK:
            wq = fp8_pack(np.asarray(W_Q[h0:h0 + NH], np.float32)
                          .reshape(NH * DH, D).T * SW)
            wk = fp8_pack(np.asarray(W_K[h0:h0 + NH], np.float32)
                          .reshape(NH * DH, D).T * SW)
        else:
            wq = chunked((np.asarray(W_Q[h0:h0 + NH], np.float32) * scale)
                         .reshape(NH * DH, D).T).astype(bf)
            wk = chunked(np.asarray(W_K[h0:h0 + NH], np.float32)
                         .reshape(NH * DH, D).T).astype(bf)
        wv = chunked(np.asarray(W_V[h0:h0 + NH], np.float32)
                     .reshape(NH * DH, D).T).astype(bf)
        wo_flat = np.asarray(W_O[h0:h0 + NH], np.float32) \
            .transpose(0, 2, 1).reshape(NH * DH, D)
        wo = np.ascontiguousarray(
            wo_flat.reshape(2, P, D).transpose(1, 0, 2)
            .reshape(P, 2 * D)).astype(bf)
        bq = np.ascontiguousarray(
            (np.asarray(b_Q[h0:h0 + NH], np.float32) * scale)
            .reshape(2, P).T)
        m = {
            "xt": xts[b], "wq": wq, "wk": wk, "wv": wv, "wo": wo,
            "bq": bq, "tri": tri,
        }
        if FP8QK:
            m["xq"] = xqs[b]
        in_maps.append(m)
    return in_maps


def run_spmd(in_maps, **kwargs):
    from concourse import bass_utils
    nc = _get_nc()
    return bass_utils.run_bass_kernel_spmd(
        nc, in_maps, core_ids=list(range(NCORES)), **kwargs)


def kernel(x, W_Q, W_K, W_V, W_O, b_Q, b_K, b_V, b_O):
    in_maps = _host_inputs(x, W_Q, W_K, W_V, W_O, b_Q, b_K, b_V)
    res = run_spmd(in_maps)
    parts = [res.results[c]["out"] for c in range(NCORES)]
    gpb = NCORES // B
    out = np.stack(
        [sum(parts[b * gpb + g] for g in range(gpb)) for b in range(B)], axis=0)
    # b_V shifts every z_h by a constant vector (softmax weights sum to 1),
    # so its whole output contribution is sum_h W_O[h] @ b_V[h]; b_K cancels
    # in the softmax entirely.
    corr = np.einsum("hdk,hk->d", np.asarray(W_O, np.float32),
                     np.asarray(b_V, np.float32))
    out += (np.asarray(b_O, np.float32) + corr)[None, None, :]
    return out.astype(np.float32)
